# revision 6
# baseline (speedup 1.0000x reference)
"""Causal self-attention (B=2,T=2048,C=1024,H=16) on 8 trn2 cores.

Head-parallel tensor parallelism: core i owns heads (2i, 2i+1).
 - QKV projection computed in transposed layout qT/kT [head_dim*2, tokens];
   V in token-major layout [tokens, 130] with a trailing ones column per head so the
   PV matmul also produces softmax denominators (M=65 per head).
 - Scores computed transposed ([tk, tq]) so softmax-normalized attention
   feeds the PV and out-proj matmuls without any transposes.
 - Causal mask via gpsimd affine_select zero-fill on diagonal tiles.
 - Out projection re-shards head-split -> token-split with two AllToAlls
   (one per batch element, so the first hides under batch-1 compute); each
   core then runs the full-C out_proj for its 512 tokens.
"""

import sys

sys.path.insert(0, "/opt/trn_rl_repo")

import numpy as np
import os

DEBUG_DUMPS = bool(int(os.environ.get("KBG", "0")))

B, T, C, H = 2, 2048, 1024, 16
D = C // H           # 64
BT = B * T           # 4096
N_CORES = 8
CHUNK = 512          # token chunk (q side)
KT = 128             # k-tile (tk side)
SLICE = BT // N_CORES  # 512 output tokens per core

_CACHE = {}


def _build_nc():
    import concourse.mybir as mybir
    import concourse.tile as tile
    from concourse import bacc
    from concourse.bass import ds, ts

    FP = mybir.dt.float32
    AF = mybir.ActivationFunctionType
    ALU = mybir.AluOpType

    nc = bacc.Bacc("TRN2", target_bir_lowering=False, debug=False,
                   num_devices=N_CORES)

    xt_d = nc.declare_dram_parameter("xt", [C, BT], FP, isOutput=False)
    wqk_d = nc.declare_dram_parameter("wqk", [C, 256], FP, isOutput=False)
    wv_d = nc.declare_dram_parameter("wv", [C, 128], FP, isOutput=False)
    bq_d = nc.declare_dram_parameter("bq", [128, 1], FP, isOutput=False)
    bk_d = nc.declare_dram_parameter("bk", [128, 1], FP, isOutput=False)
    bv_d = nc.declare_dram_parameter("bv", [1, 128], FP, isOutput=False)
    wout_d = nc.declare_dram_parameter("wout", [C, C], FP, isOutput=False)
    bout_d = nc.declare_dram_parameter("bout_bc", [128, C], FP, isOutput=False)
    y_d = nc.declare_dram_parameter("y", [SLICE, C], FP, isOutput=True)
    dbg = {}
    if DEBUG_DUMPS:
        for nm, shape in [("dbg_qt", [128, 512]), ("dbg_kt", [128, 512]),
                          ("dbg_v", [128, 130]), ("dbg_ex", [128, 1024]),
                          ("dbg_out", [128, 512]), ("dbg_pin", [128, 8, 256]),
                          ("dbg_a2ain", [8, 128, 256]), ("dbg_a2aout", [8, 128, 256])]:
            dbg[nm] = nc.declare_dram_parameter(nm, shape, FP, isOutput=True)

    with tile.TileContext(nc) as tc:
        with (
            tc.tile_pool(name="wpool", bufs=1) as wpool,
            tc.tile_pool(name="xtp", bufs=2) as xtp,
            tc.tile_pool(name="qkp", bufs=1) as qkp,
            tc.tile_pool(name="vp", bufs=32) as vp,
            tc.tile_pool(name="outp", bufs=1) as outp,
            tc.tile_pool(name="expp", bufs=6) as expp,
            tc.tile_pool(name="recp", bufs=2) as recp,
            tc.tile_pool(name="projp", bufs=1) as projp,
            tc.tile_pool(name="yp", bufs=1) as yp,
            tc.tile_pool(name="psb", bufs=3, space="PSUM") as psb,
            tc.tile_pool(name="psa", bufs=2, space="PSUM") as psa,
            tc.tile_pool(name="dram", bufs=1, space="DRAM") as dram,
        ):
            # resident weights / constants
            wqk_sb = wpool.tile([128, 8, 256], FP)
            nc.sync.dma_start(out=wqk_sb[:], in_=wqk_d.ap().rearrange("(k p) m -> p k m", p=128))
            wv_sb = wpool.tile([128, 8, 128], FP)
            nc.sync.dma_start(out=wv_sb[:], in_=wv_d.ap().rearrange("(k p) m -> p k m", p=128))
            wout_sb = wpool.tile([128, 8, 1024], FP)
            nc.sync.dma_start(out=wout_sb[:], in_=wout_d.ap().rearrange("(i p) n -> p i n", p=128))
            bq_sb = wpool.tile([128, 1], FP)
            nc.sync.dma_start(out=bq_sb[:], in_=bq_d.ap())
            bk_sb = wpool.tile([128, 1], FP)
            nc.sync.dma_start(out=bk_sb[:], in_=bk_d.ap())
            bv_sb = wpool.tile([1, 128], FP)
            nc.sync.dma_start(out=bv_sb[:], in_=bv_d.ap())
            bout_sb = wpool.tile([128, 1024], FP)
            nc.sync.dma_start(out=bout_sb[:], in_=bout_d.ap())
            ones128 = wpool.tile([1, 128], FP)
            nc.vector.memset(ones128[:], 1.0)
            ones64 = wpool.tile([1, 64], FP)
            nc.vector.memset(ones64[:], 1.0)

            # resident activations (per 512-token chunk tiles)
            qT = [qkp.tile([128, CHUNK], FP, name=f"qT{g}") for g in range(8)]
            kT = [qkp.tile([128, CHUNK], FP, name=f"kT{g}") for g in range(8)]
            outT = [outp.tile([128, CHUNK], FP, name=f"outT{g}") for g in range(8)]
            v_tiles = [None] * 32

            def qkv_chunk(g):
                """Project tokens [512g, 512g+512) -> qT/kT chunk g, v tiles."""
                xt_t = xtp.tile([128, 8, CHUNK], FP, name="xt_t")
                nc.sync.dma_start(
                    out=xt_t[:],
                    in_=xt_d.ap()[:, ds(CHUNK * g, CHUNK)].rearrange("(k p) t -> p k t", p=128),
                )
                for m in (0, 1):  # 0 -> q, 1 -> k
                    ps = psb.tile([128, CHUNK], FP, tag="big", name="ps_qk")
                    for k in range(8):
                        nc.tensor.matmul(ps[:], wqk_sb[:, k, ts(m, 128)], xt_t[:, k, :],
                                         start=(k == 0), stop=(k == 7))
                    if m == 0:
                        nc.vector.tensor_scalar(out=qT[g][:], in0=ps[:], scalar1=0.125,
                                                scalar2=bq_sb[:], op0=ALU.mult, op1=ALU.add)
                    else:
                        nc.vector.tensor_scalar(out=kT[g][:], in0=ps[:], scalar1=bk_sb[:],
                                                scalar2=None, op0=ALU.add)
                for mt in range(4):  # v token sub-tiles of 128
                    ti = 4 * g + mt
                    psv = psb.tile([128, 128], FP, tag="big", name="ps_v")
                    for k in range(8):
                        nc.tensor.matmul(psv[:], xt_t[:, k, ts(mt, 128)], wv_sb[:, k, :],
                                         start=(k == 0), stop=False)
                    nc.tensor.matmul(psv[:], ones128[:], bv_sb[:], start=False, stop=True)
                    vt = vp.tile([128, 130], FP, name="vt")
                    nc.vector.memset(vt[:, 64:65], 1.0)
                    nc.vector.memset(vt[:, 129:130], 1.0)
                    nc.vector.tensor_copy(vt[:, 0:64], psv[:, 0:64])
                    nc.vector.tensor_copy(vt[:, 65:129], psv[:, 64:128])
                    v_tiles[ti] = vt

            def attn_chunk(b, c):
                """Attention for q tokens chunk c of batch b, both heads."""
                g = 4 * b + c
                nkt = 4 * c + 4  # causal: k-tiles 0 .. 4c+3 (local to b)
                acc = [psa.tile([65, CHUNK], FP, tag="acc", name=f"acc{h}") for h in (0, 1)]
                q_sl = [qT[g][0:64, :], qT[g][64:128, :]]
                for grp in range(nkt // 2):
                    ps = [psb.tile([128, 1024], FP, tag="big", name=f"ps_s{h}") for h in (0, 1)]
                    for kk in (0, 1):
                        kt = 2 * grp + kk
                        kg = 4 * b + kt // 4   # qk chunk holding this k-tile
                        ksl = ts(kt % 4, 128)
                        # both heads adjacent -> concurrent on PE (rows 0-63 / 64-127)
                        nc.tensor.matmul(ps[0][:, ts(kk, CHUNK)], kT[kg][0:64, ksl], q_sl[0],
                                         start=True, stop=True)
                        nc.tensor.matmul(ps[1][:, ts(kk, CHUNK)], kT[kg][64:128, ksl], q_sl[1],
                                         start=True, stop=True)
                    ex = [expp.tile([128, 1024], FP, name=f"ex{h}", tag="ex") for h in (0, 1)]
                    for h in (0, 1):
                        nc.scalar.activation(ex[h][:], ps[h][:], AF.Exp)
                    if DEBUG_DUMPS and b == 0 and c == 0 and grp == 0:
                        nc.sync.dma_start(out=dbg["dbg_ex"].ap(), in_=ex[0][:])
                    for kk in (0, 1):
                        kt = 2 * grp + kk
                        j = kt - 4 * c
                        if j >= 0:  # diagonal tile: zero where tq < tk
                            for h in (0, 1):
                                nc.gpsimd.affine_select(
                                    out=ex[h][:, ts(kk, CHUNK)], in_=ex[h][:, ts(kk, CHUNK)],
                                    pattern=[[1, CHUNK]], compare_op=ALU.is_ge,
                                    fill=0.0, base=-128 * j, channel_multiplier=-1,
                                )
                    for kk in (0, 1):
                        kt = 2 * grp + kk
                        vt = v_tiles[16 * b + kt]
                        for h in (0, 1):
                            nc.tensor.matmul(acc[h][:], vt[:, 65 * h:65 * h + 65],
                                             ex[h][:, ts(kk, CHUNK)],
                                             start=(kt == 0), stop=(kt == nkt - 1))
                # normalize: out[d, tq] * (1 / sum[tq]), write into outT chunk
                for h in (0, 1):
                    rec = recp.tile([1, CHUNK], FP, name="rec")
                    nc.vector.reciprocal(rec[:], acc[h][64:65, :])
                    bc = psb.tile([64, CHUNK], FP, tag="big", name="bc")
                    nc.tensor.matmul(bc[:], ones64[:], rec[:], start=True, stop=True)
                    dst = outT[g][64 * h:64 * h + 64, :]
                    nc.vector.tensor_copy(dst, acc[h][0:64, :])
                    nc.vector.tensor_mul(dst, dst, bc[:])

            def proj_half(b):
                """AllToAll re-shard (head-split -> token-split) + out_proj for
                this core's 256 tokens of batch b."""
                a2a_in = dram.tile([8, 128, 256], FP, name=f"a2a_in{b}")
                for cc in range(4):
                    nc.sync.dma_start(
                        out=a2a_in[2 * cc:2 * cc + 2, :, :].rearrange("j p t -> p j t"),
                        in_=outT[4 * b + cc].rearrange("p (j t) -> p j t", j=2),
                    )
                a2a_out = dram.tile([8, 128, 256], FP, name=f"a2a_out{b}")
                nc.gpsimd.collective_compute(
                    "AllToAll", mybir.AluOpType.bypass,
                    replica_groups=[list(range(N_CORES))],
                    ins=[a2a_in.opt()], outs=[a2a_out.opt()],
                )
                pin = projp.tile([128, 8, 256], FP, name="pin")
                nc.sync.dma_start(out=pin[:], in_=a2a_out.rearrange("j p t -> p j t"))
                if DEBUG_DUMPS and b == 0:
                    nc.sync.dma_start(out=dbg["dbg_pin"].ap(), in_=pin[:])
                    nc.sync.dma_start(out=dbg["dbg_a2ain"].ap(), in_=a2a_in[:])
                    nc.sync.dma_start(out=dbg["dbg_a2aout"].ap(), in_=a2a_out[:])
                for m in (0, 1):
                    y_sb = yp.tile([128, 1024], FP, name="y_sb")
                    for n in (0, 1):
                        psy = psb.tile([128, CHUNK], FP, tag="big", name="psy")
                        for i in range(8):
                            nc.tensor.matmul(psy[:], pin[:, i, ts(m, 128)],
                                             wout_sb[:, i, ts(n, CHUNK)],
                                             start=(i == 0), stop=(i == 7))
                        nc.vector.tensor_add(y_sb[:, ts(n, CHUNK)], psy[:], bout_sb[:, ts(n, CHUNK)])
                    nc.sync.dma_start(out=y_d.ap()[ds(256 * b + 128 * m, 128), :], in_=y_sb[:])

            for b in range(B):
                for c in range(4):
                    qkv_chunk(4 * b + c)
                    attn_chunk(b, c)
                proj_half(b)
            if DEBUG_DUMPS:
                nc.sync.dma_start(out=dbg["dbg_qt"].ap(), in_=qT[0][:])
                nc.sync.dma_start(out=dbg["dbg_kt"].ap(), in_=kT[0][:])
                nc.sync.dma_start(out=dbg["dbg_v"].ap(), in_=v_tiles[0][:])
                nc.sync.dma_start(out=dbg["dbg_out"].ap(), in_=outT[0][:])

    nc.compile()
    return nc


def _get_runner():
    if "runner" in _CACHE:
        return _CACHE["runner"]

    import jax
    from jax.sharding import Mesh, PartitionSpec
    from jax.experimental.shard_map import shard_map
    from concourse import bass2jax
    from concourse.bass2jax import _bass_exec_p, partition_id_tensor
    import concourse.mybir as mybir

    bass2jax.install_neuronx_cc_hook()
    nc = _build_nc()

    in_names, out_names, out_avals, zero_shapes = [], [], [], []
    for alloc in nc.m.functions[0].allocations:
        if not isinstance(alloc, mybir.MemoryLocationSet):
            continue
        name = alloc.memorylocations[0].name
        pname = nc.partition_id_tensor.name if nc.partition_id_tensor else None
        if alloc.kind == "ExternalInput" and name != pname:
            in_names.append(name)
        elif alloc.kind == "ExternalOutput":
            out_names.append(name)
            shape = tuple(alloc.tensor_shape)
            dtype = mybir.dt.np(alloc.dtype)
            out_avals.append(jax.core.ShapedArray(shape, dtype))
            zero_shapes.append((shape, dtype))
    n_params = len(in_names)
    all_in_names = list(in_names) + list(out_names)
    partition_name = nc.partition_id_tensor.name if nc.partition_id_tensor else None
    if partition_name is not None:
        all_in_names.append(partition_name)

    def _body(*args):
        operands = list(args)
        if partition_name is not None:
            operands.append(partition_id_tensor())
        outs = _bass_exec_p.bind(
            *operands,
            out_avals=tuple(out_avals),
            in_names=tuple(all_in_names),
            out_names=tuple(out_names),
            lowering_input_output_aliases=(),
            sim_require_finite=True,
            sim_require_nnan=True,
            nc=nc,
        )
        return tuple(outs)

    devices = jax.devices()[:N_CORES]
    mesh = Mesh(np.asarray(devices), ("core",))
    n_outs = len(out_names)
    sharded = jax.jit(
        shard_map(_body, mesh=mesh,
                  in_specs=(PartitionSpec("core"),) * (n_params + n_outs),
                  out_specs=(PartitionSpec("core"),) * n_outs,
                  check_rep=False),
        donate_argnums=tuple(range(n_params, n_params + n_outs)),
        keep_unused=True,
    )

    runner = {
        "sharded": sharded,
        "in_names": in_names,
        "out_names": out_names,
        "zero_shapes": zero_shapes,
        "n_params": n_params,
    }
    _CACHE["runner"] = runner
    return runner


def _shard_inputs(x, Wqkv, bqkv, Wout, bout):
    """Build per-core input dicts (host-side sharding / layout prep)."""
    x2 = np.asarray(x, np.float32).reshape(BT, C)
    xt = np.ascontiguousarray(x2.T)
    Wqkv = np.asarray(Wqkv, np.float32)
    bqkv = np.asarray(bqkv, np.float32)
    Wout = np.ascontiguousarray(np.asarray(Wout, np.float32))
    bout = np.asarray(bout, np.float32)
    bout_bc = np.ascontiguousarray(np.broadcast_to(bout, (128, C)))

    in_maps = []
    for i in range(N_CORES):
        h0, h1 = 2 * i, 2 * i + 1
        cols = lambda s, h: slice(s * C + h * D, s * C + (h + 1) * D)
        wqk = np.concatenate(
            [Wqkv[:, cols(0, h0)], Wqkv[:, cols(0, h1)],
             Wqkv[:, cols(1, h0)], Wqkv[:, cols(1, h1)]], axis=1)
        wv = np.concatenate([Wqkv[:, cols(2, h0)], Wqkv[:, cols(2, h1)]], axis=1)
        bq = np.concatenate([bqkv[cols(0, h0)], bqkv[cols(0, h1)]]) / 8.0
        bk = np.concatenate([bqkv[cols(1, h0)], bqkv[cols(1, h1)]])
        bv = np.concatenate([bqkv[cols(2, h0)], bqkv[cols(2, h1)]])
        in_maps.append({
            "xt": xt,
            "wqk": np.ascontiguousarray(wqk),
            "wv": np.ascontiguousarray(wv),
            "bq": np.ascontiguousarray(bq.reshape(128, 1)),
            "bk": np.ascontiguousarray(bk.reshape(128, 1)),
            "bv": np.ascontiguousarray(bv.reshape(1, 128)),
            "wout": Wout,
            "bout_bc": bout_bc,
        })
    return in_maps


def _concat_inputs(runner, in_maps):
    return [
        np.concatenate([np.asarray(in_maps[c][name]) for c in range(N_CORES)], axis=0)
        for name in runner["in_names"]
    ]


def _run(runner, concat_in):
    zeros = [np.zeros((N_CORES * s[0], *s[1:]), dt) for s, dt in runner["zero_shapes"]]
    out_arrs = runner["sharded"](*concat_in, *zeros)
    return [np.asarray(a) for a in out_arrs]


def _gather(runner, outs):
    y = outs[runner["out_names"].index("y")].reshape(N_CORES, SLICE, C)
    full = np.empty((BT, C), np.float32)
    for j in range(N_CORES):
        full[256 * j:256 * j + 256] = y[j, :256]
        full[T + 256 * j:T + 256 * j + 256] = y[j, 256:]
    return full.reshape(B, T, C)


def kernel(x, Wqkv, bqkv, Wout, bout):
    runner = _get_runner()
    in_maps = _shard_inputs(x, Wqkv, bqkv, Wout, bout)
    outs = _run(runner, _concat_inputs(runner, in_maps))
    return _gather(runner, outs)


if __name__ == "__main__":
    rng = np.random.default_rng(0)
    x = rng.standard_normal((B, T, C), np.float32)
    Wqkv = (rng.standard_normal((C, 3 * C)) * 0.02).astype(np.float32)
    bqkv = np.zeros(3 * C, np.float32)
    Wout = (rng.standard_normal((C, C)) * 0.02).astype(np.float32)
    bout = np.zeros(C, np.float32)
    y = kernel(x, Wqkv, bqkv, Wout, bout)
    print("kernel ran:", y.shape, y.dtype, float(np.abs(y).max()))


# revision 8
# speedup vs baseline: 1.9625x; 1.9625x over previous
"""Causal self-attention (B=2,T=2048,C=1024,H=16) on 8 trn2 cores.

Head-parallel tensor parallelism: core i owns heads (2i, 2i+1).
 - QKV projection computed in transposed layout qT/kT [head_dim*2, tokens];
   V in token-major layout [tokens, 130] with a trailing ones column per head so the
   PV matmul also produces softmax denominators (M=65 per head).
 - Scores computed transposed ([tk, tq]) so softmax-normalized attention
   feeds the PV and out-proj matmuls without any transposes.
 - Causal mask via gpsimd affine_select zero-fill on diagonal tiles.
 - Out projection re-shards head-split -> token-split with two AllToAlls
   (one per batch element, so the first hides under batch-1 compute); each
   core then runs the full-C out_proj for its 512 tokens.
"""

import sys

sys.path.insert(0, "/opt/trn_rl_repo")

import numpy as np
import os

DEBUG_DUMPS = bool(int(os.environ.get("KBG", "0")))
KSTAGE = int(os.environ.get("KSTAGE", "3"))
KA2A = int(os.environ.get("KA2A", "2"))

B, T, C, H = 2, 2048, 1024, 16
D = C // H           # 64
BT = B * T           # 4096
N_CORES = 8
CHUNK = 512          # token chunk (q side)
KT = 128             # k-tile (tk side)
SLICE = BT // N_CORES  # 512 output tokens per core

_CACHE = {}


def _build_nc():
    import concourse.mybir as mybir
    import concourse.tile as tile
    from concourse import bacc
    from concourse.bass import ds, ts

    FP = mybir.dt.float32
    AF = mybir.ActivationFunctionType
    ALU = mybir.AluOpType

    nc = bacc.Bacc("TRN2", target_bir_lowering=False, debug=False,
                   num_devices=N_CORES)

    xt_d = nc.declare_dram_parameter("xt", [C, BT], FP, isOutput=False)
    wqk_d = nc.declare_dram_parameter("wqk", [C, 256], FP, isOutput=False)
    wv_d = nc.declare_dram_parameter("wv", [C, 128], FP, isOutput=False)
    bq_d = nc.declare_dram_parameter("bq", [128, 1], FP, isOutput=False)
    bk_d = nc.declare_dram_parameter("bk", [128, 1], FP, isOutput=False)
    bv_d = nc.declare_dram_parameter("bv", [1, 128], FP, isOutput=False)
    wout_d = nc.declare_dram_parameter("wout", [C, C], FP, isOutput=False)
    bout_d = nc.declare_dram_parameter("bout_bc", [128, C], FP, isOutput=False)
    y_d = nc.declare_dram_parameter("y", [SLICE, C], FP, isOutput=True)
    dbg = {}
    if DEBUG_DUMPS:
        for nm, shape in [("dbg_qt", [128, 512]), ("dbg_kt", [128, 512]),
                          ("dbg_v", [128, 130]), ("dbg_ex", [128, 1024]),
                          ("dbg_out", [128, 512]), ("dbg_pin", [128, 8, 256]),
                          ("dbg_a2ain", [8, 128, 256]), ("dbg_a2aout", [8, 128, 256])]:
            dbg[nm] = nc.declare_dram_parameter(nm, shape, FP, isOutput=True)

    with tile.TileContext(nc) as tc:
        with (
            tc.tile_pool(name="wpool", bufs=1) as wpool,
            tc.tile_pool(name="xtp", bufs=2) as xtp,
            tc.tile_pool(name="qkp", bufs=1) as qkp,
            tc.tile_pool(name="vp", bufs=32) as vp,
            tc.tile_pool(name="outp", bufs=1) as outp,
            tc.tile_pool(name="expp", bufs=6) as expp,
            tc.tile_pool(name="recp", bufs=2) as recp,
            tc.tile_pool(name="projp", bufs=1) as projp,
            tc.tile_pool(name="yp", bufs=1) as yp,
            tc.tile_pool(name="psb", bufs=3, space="PSUM") as psb,
            tc.tile_pool(name="psa", bufs=2, space="PSUM") as psa,
            tc.tile_pool(name="dram", bufs=1, space="DRAM") as dram,
        ):
            # resident weights / constants
            wqk_sb = wpool.tile([128, 8, 256], FP)
            nc.sync.dma_start(out=wqk_sb[:], in_=wqk_d.ap().rearrange("(k p) m -> p k m", p=128))
            wv_sb = wpool.tile([128, 8, 128], FP)
            nc.sync.dma_start(out=wv_sb[:], in_=wv_d.ap().rearrange("(k p) m -> p k m", p=128))
            wout_sb = wpool.tile([128, 8, 1024], FP)
            nc.sync.dma_start(out=wout_sb[:], in_=wout_d.ap().rearrange("(i p) n -> p i n", p=128))
            bq_sb = wpool.tile([128, 1], FP)
            nc.sync.dma_start(out=bq_sb[:], in_=bq_d.ap())
            bk_sb = wpool.tile([128, 1], FP)
            nc.sync.dma_start(out=bk_sb[:], in_=bk_d.ap())
            bv_sb = wpool.tile([1, 128], FP)
            nc.sync.dma_start(out=bv_sb[:], in_=bv_d.ap())
            bout_sb = wpool.tile([128, 1024], FP)
            nc.sync.dma_start(out=bout_sb[:], in_=bout_d.ap())
            ones128 = wpool.tile([1, 128], FP)
            nc.vector.memset(ones128[:], 1.0)
            ones64 = wpool.tile([1, 64], FP)
            nc.vector.memset(ones64[:], 1.0)

            # resident activations (per 512-token chunk tiles)
            qT = [qkp.tile([128, CHUNK], FP, name=f"qT{g}") for g in range(8)]
            kT = [qkp.tile([128, CHUNK], FP, name=f"kT{g}") for g in range(8)]
            outT = [outp.tile([128, CHUNK], FP, name=f"outT{g}") for g in range(8)]
            v_tiles = [None] * 32

            def qkv_chunk(g):
                """Project tokens [512g, 512g+512) -> qT/kT chunk g, v tiles."""
                xt_t = xtp.tile([128, 8, CHUNK], FP, name="xt_t")
                nc.sync.dma_start(
                    out=xt_t[:],
                    in_=xt_d.ap()[:, ds(CHUNK * g, CHUNK)].rearrange("(k p) t -> p k t", p=128),
                )
                for m in (0, 1):  # 0 -> q, 1 -> k
                    ps = psb.tile([128, CHUNK], FP, tag="big", name="ps_qk")
                    for k in range(8):
                        nc.tensor.matmul(ps[:], wqk_sb[:, k, ts(m, 128)], xt_t[:, k, :],
                                         start=(k == 0), stop=(k == 7))
                    if m == 0:
                        nc.vector.tensor_scalar(out=qT[g][:], in0=ps[:], scalar1=0.125,
                                                scalar2=bq_sb[:], op0=ALU.mult, op1=ALU.add)
                    else:
                        nc.vector.tensor_scalar(out=kT[g][:], in0=ps[:], scalar1=bk_sb[:],
                                                scalar2=None, op0=ALU.add)
                for mt in range(4):  # v token sub-tiles of 128
                    ti = 4 * g + mt
                    psv = psb.tile([128, 128], FP, tag="big", name="ps_v")
                    for k in range(8):
                        nc.tensor.matmul(psv[:], xt_t[:, k, ts(mt, 128)], wv_sb[:, k, :],
                                         start=(k == 0), stop=False)
                    nc.tensor.matmul(psv[:], ones128[:], bv_sb[:], start=False, stop=True)
                    vt = vp.tile([128, 130], FP, name="vt")
                    nc.vector.memset(vt[:, 64:65], 1.0)
                    nc.vector.memset(vt[:, 129:130], 1.0)
                    nc.vector.tensor_copy(vt[:, 0:64], psv[:, 0:64])
                    nc.vector.tensor_copy(vt[:, 65:129], psv[:, 64:128])
                    v_tiles[ti] = vt

            def attn_chunk(b, c):
                """Attention for q tokens chunk c of batch b, both heads."""
                g = 4 * b + c
                nkt = 4 * c + 4  # causal: k-tiles 0 .. 4c+3 (local to b)
                acc = [psa.tile([65, CHUNK], FP, tag="acc", name=f"acc{h}") for h in (0, 1)]
                q_sl = [qT[g][0:64, :], qT[g][64:128, :]]
                for grp in range(nkt // 2):
                    ps = [psb.tile([128, 1024], FP, tag="big", name=f"ps_s{h}") for h in (0, 1)]
                    for kk in (0, 1):
                        kt = 2 * grp + kk
                        kg = 4 * b + kt // 4   # qk chunk holding this k-tile
                        ksl = ts(kt % 4, 128)
                        # both heads adjacent -> concurrent on PE (rows 0-63 / 64-127)
                        nc.tensor.matmul(ps[0][:, ts(kk, CHUNK)], kT[kg][0:64, ksl], q_sl[0],
                                         start=True, stop=True)
                        nc.tensor.matmul(ps[1][:, ts(kk, CHUNK)], kT[kg][64:128, ksl], q_sl[1],
                                         start=True, stop=True)
                    ex = [expp.tile([128, 1024], FP, name=f"ex{h}", tag="ex") for h in (0, 1)]
                    for h in (0, 1):
                        nc.scalar.activation(ex[h][:], ps[h][:], AF.Exp)
                    if DEBUG_DUMPS and b == 0 and c == 0 and grp == 0:
                        nc.sync.dma_start(out=dbg["dbg_ex"].ap(), in_=ex[0][:])
                    for kk in (0, 1):
                        kt = 2 * grp + kk
                        j = kt - 4 * c
                        if j >= 0:  # diagonal tile: zero where tq < tk
                            for h in (0, 1):
                                nc.gpsimd.affine_select(
                                    out=ex[h][:, ts(kk, CHUNK)], in_=ex[h][:, ts(kk, CHUNK)],
                                    pattern=[[1, CHUNK]], compare_op=ALU.is_ge,
                                    fill=0.0, base=-128 * j, channel_multiplier=-1,
                                )
                    for kk in (0, 1):
                        kt = 2 * grp + kk
                        vt = v_tiles[16 * b + kt]
                        for h in (0, 1):
                            nc.tensor.matmul(acc[h][:], vt[:, 65 * h:65 * h + 65],
                                             ex[h][:, ts(kk, CHUNK)],
                                             start=(kt == 0), stop=(kt == nkt - 1))
                # normalize: out[d, tq] * (1 / sum[tq]), write into outT chunk
                for h in (0, 1):
                    rec = recp.tile([1, CHUNK], FP, name="rec")
                    nc.vector.reciprocal(rec[:], acc[h][64:65, :])
                    bc = psb.tile([64, CHUNK], FP, tag="big", name="bc")
                    nc.tensor.matmul(bc[:], ones64[:], rec[:], start=True, stop=True)
                    dst = outT[g][64 * h:64 * h + 64, :]
                    nc.vector.tensor_copy(dst, acc[h][0:64, :])
                    nc.vector.tensor_mul(dst, dst, bc[:])

            def proj_single():
                """One AllToAll over all 4096 tokens; core j -> tokens [512j, 512j+512)."""
                a2a_in = dram.tile([8, 128, CHUNK], FP, name="a2a_in")
                for g in range(8):
                    nc.sync.dma_start(out=a2a_in[g], in_=outT[g][:])
                a2a_out = dram.tile([8, 128, CHUNK], FP, name="a2a_out")
                nc.gpsimd.collective_compute(
                    "AllToAll", mybir.AluOpType.bypass,
                    replica_groups=[list(range(N_CORES))],
                    ins=[a2a_in.opt()], outs=[a2a_out.opt()],
                )
                pin = projp.tile([128, 8, CHUNK], FP, name="pin")
                nc.sync.dma_start(out=pin[:], in_=a2a_out.rearrange("j p t -> p j t"))
                for m in range(4):
                    y_sb = yp.tile([128, 1024], FP, name="y_sb")
                    for n in (0, 1):
                        psy = psb.tile([128, CHUNK], FP, tag="big", name="psy")
                        for i in range(8):
                            nc.tensor.matmul(psy[:], pin[:, i, ts(m, 128)],
                                             wout_sb[:, i, ts(n, CHUNK)],
                                             start=(i == 0), stop=(i == 7))
                        nc.vector.tensor_add(y_sb[:, ts(n, CHUNK)], psy[:], bout_sb[:, ts(n, CHUNK)])
                    nc.sync.dma_start(out=y_d.ap()[ds(128 * m, 128), :], in_=y_sb[:])

            def proj_half(b):
                """AllToAll re-shard (head-split -> token-split) + out_proj for
                this core's 256 tokens of batch b."""
                a2a_in = dram.tile([8, 128, 256], FP, name=f"a2a_in{b}")
                for cc in range(4):
                    nc.sync.dma_start(
                        out=a2a_in[2 * cc:2 * cc + 2, :, :].rearrange("j p t -> p j t"),
                        in_=outT[4 * b + cc].rearrange("p (j t) -> p j t", j=2),
                    )
                a2a_out = dram.tile([8, 128, 256], FP, name=f"a2a_out{b}")
                nc.gpsimd.collective_compute(
                    "AllToAll", mybir.AluOpType.bypass,
                    replica_groups=[list(range(N_CORES))],
                    ins=[a2a_in.opt()], outs=[a2a_out.opt()],
                )
                pin = projp.tile([128, 8, 256], FP, name="pin")
                nc.sync.dma_start(out=pin[:], in_=a2a_out.rearrange("j p t -> p j t"))
                if DEBUG_DUMPS and b == 0:
                    nc.sync.dma_start(out=dbg["dbg_pin"].ap(), in_=pin[:])
                    nc.sync.dma_start(out=dbg["dbg_a2ain"].ap(), in_=a2a_in[:])
                    nc.sync.dma_start(out=dbg["dbg_a2aout"].ap(), in_=a2a_out[:])
                for m in (0, 1):
                    y_sb = yp.tile([128, 1024], FP, name="y_sb")
                    for n in (0, 1):
                        psy = psb.tile([128, CHUNK], FP, tag="big", name="psy")
                        for i in range(8):
                            nc.tensor.matmul(psy[:], pin[:, i, ts(m, 128)],
                                             wout_sb[:, i, ts(n, CHUNK)],
                                             start=(i == 0), stop=(i == 7))
                        nc.vector.tensor_add(y_sb[:, ts(n, CHUNK)], psy[:], bout_sb[:, ts(n, CHUNK)])
                    nc.sync.dma_start(out=y_d.ap()[ds(256 * b + 128 * m, 128), :], in_=y_sb[:])

            for b in range(B):
                for c in range(4):
                    if KSTAGE >= 1:
                        qkv_chunk(4 * b + c)
                    if KSTAGE >= 2:
                        attn_chunk(b, c)
                if KSTAGE >= 3 and KA2A == 2:
                    proj_half(b)
            if KSTAGE >= 3 and KA2A == 1:
                proj_single()
            if KSTAGE < 3:
                # still produce the output so the program shape is unchanged
                for m in range(4):
                    fill = yp.tile([128, 1024], FP, name="y_sb")
                    src = outT[m] if KSTAGE >= 2 else wout_sb[:, 0, :]
                    nc.vector.tensor_copy(fill[:, 0:512], src[:, 0:512])
                    nc.vector.tensor_copy(fill[:, 512:1024], src[:, 0:512])
                    nc.sync.dma_start(out=y_d.ap()[ds(128 * m, 128), :], in_=fill[:])
            if DEBUG_DUMPS:
                nc.sync.dma_start(out=dbg["dbg_qt"].ap(), in_=qT[0][:])
                nc.sync.dma_start(out=dbg["dbg_kt"].ap(), in_=kT[0][:])
                nc.sync.dma_start(out=dbg["dbg_v"].ap(), in_=v_tiles[0][:])
                nc.sync.dma_start(out=dbg["dbg_out"].ap(), in_=outT[0][:])

    nc.compile()
    return nc


def _get_runner():
    if "runner" in _CACHE:
        return _CACHE["runner"]

    import jax
    from jax.sharding import Mesh, PartitionSpec
    from jax.experimental.shard_map import shard_map
    from concourse import bass2jax
    from concourse.bass2jax import _bass_exec_p, partition_id_tensor
    import concourse.mybir as mybir

    bass2jax.install_neuronx_cc_hook()
    nc = _build_nc()

    in_names, out_names, out_avals, zero_shapes = [], [], [], []
    for alloc in nc.m.functions[0].allocations:
        if not isinstance(alloc, mybir.MemoryLocationSet):
            continue
        name = alloc.memorylocations[0].name
        pname = nc.partition_id_tensor.name if nc.partition_id_tensor else None
        if alloc.kind == "ExternalInput" and name != pname:
            in_names.append(name)
        elif alloc.kind == "ExternalOutput":
            out_names.append(name)
            shape = tuple(alloc.tensor_shape)
            dtype = mybir.dt.np(alloc.dtype)
            out_avals.append(jax.core.ShapedArray(shape, dtype))
            zero_shapes.append((shape, dtype))
    n_params = len(in_names)
    all_in_names = list(in_names) + list(out_names)
    partition_name = nc.partition_id_tensor.name if nc.partition_id_tensor else None
    if partition_name is not None:
        all_in_names.append(partition_name)

    def _body(*args):
        operands = list(args)
        if partition_name is not None:
            operands.append(partition_id_tensor())
        outs = _bass_exec_p.bind(
            *operands,
            out_avals=tuple(out_avals),
            in_names=tuple(all_in_names),
            out_names=tuple(out_names),
            lowering_input_output_aliases=(),
            sim_require_finite=True,
            sim_require_nnan=True,
            nc=nc,
        )
        return tuple(outs)

    devices = jax.devices()[:N_CORES]
    mesh = Mesh(np.asarray(devices), ("core",))
    n_outs = len(out_names)
    sharded = jax.jit(
        shard_map(_body, mesh=mesh,
                  in_specs=(PartitionSpec("core"),) * (n_params + n_outs),
                  out_specs=(PartitionSpec("core"),) * n_outs,
                  check_rep=False),
        donate_argnums=tuple(range(n_params, n_params + n_outs)),
        keep_unused=True,
    )

    runner = {
        "sharded": sharded,
        "in_names": in_names,
        "out_names": out_names,
        "zero_shapes": zero_shapes,
        "n_params": n_params,
    }
    _CACHE["runner"] = runner
    return runner


def _shard_inputs(x, Wqkv, bqkv, Wout, bout):
    """Build per-core input dicts (host-side sharding / layout prep)."""
    x2 = np.asarray(x, np.float32).reshape(BT, C)
    xt = np.ascontiguousarray(x2.T)
    Wqkv = np.asarray(Wqkv, np.float32)
    bqkv = np.asarray(bqkv, np.float32)
    Wout = np.ascontiguousarray(np.asarray(Wout, np.float32))
    bout = np.asarray(bout, np.float32)
    bout_bc = np.ascontiguousarray(np.broadcast_to(bout, (128, C)))

    in_maps = []
    for i in range(N_CORES):
        h0, h1 = 2 * i, 2 * i + 1
        cols = lambda s, h: slice(s * C + h * D, s * C + (h + 1) * D)
        wqk = np.concatenate(
            [Wqkv[:, cols(0, h0)], Wqkv[:, cols(0, h1)],
             Wqkv[:, cols(1, h0)], Wqkv[:, cols(1, h1)]], axis=1)
        wv = np.concatenate([Wqkv[:, cols(2, h0)], Wqkv[:, cols(2, h1)]], axis=1)
        bq = np.concatenate([bqkv[cols(0, h0)], bqkv[cols(0, h1)]]) / 8.0
        bk = np.concatenate([bqkv[cols(1, h0)], bqkv[cols(1, h1)]])
        bv = np.concatenate([bqkv[cols(2, h0)], bqkv[cols(2, h1)]])
        in_maps.append({
            "xt": xt,
            "wqk": np.ascontiguousarray(wqk),
            "wv": np.ascontiguousarray(wv),
            "bq": np.ascontiguousarray(bq.reshape(128, 1)),
            "bk": np.ascontiguousarray(bk.reshape(128, 1)),
            "bv": np.ascontiguousarray(bv.reshape(1, 128)),
            "wout": Wout,
            "bout_bc": bout_bc,
        })
    return in_maps


def _concat_inputs(runner, in_maps):
    return [
        np.concatenate([np.asarray(in_maps[c][name]) for c in range(N_CORES)], axis=0)
        for name in runner["in_names"]
    ]


def _run(runner, concat_in):
    zeros = [np.zeros((N_CORES * s[0], *s[1:]), dt) for s, dt in runner["zero_shapes"]]
    out_arrs = runner["sharded"](*concat_in, *zeros)
    return [np.asarray(a) for a in out_arrs]


def _gather(runner, outs):
    y = outs[runner["out_names"].index("y")].reshape(N_CORES, SLICE, C)
    if KA2A == 1:
        return np.ascontiguousarray(y.reshape(BT, C)).reshape(B, T, C)
    full = np.empty((BT, C), np.float32)
    for j in range(N_CORES):
        full[256 * j:256 * j + 256] = y[j, :256]
        full[T + 256 * j:T + 256 * j + 256] = y[j, 256:]
    return full.reshape(B, T, C)


def kernel(x, Wqkv, bqkv, Wout, bout):
    runner = _get_runner()
    in_maps = _shard_inputs(x, Wqkv, bqkv, Wout, bout)
    outs = _run(runner, _concat_inputs(runner, in_maps))
    return _gather(runner, outs)


if __name__ == "__main__":
    rng = np.random.default_rng(0)
    x = rng.standard_normal((B, T, C), np.float32)
    Wqkv = (rng.standard_normal((C, 3 * C)) * 0.02).astype(np.float32)
    bqkv = np.zeros(3 * C, np.float32)
    Wout = (rng.standard_normal((C, C)) * 0.02).astype(np.float32)
    bout = np.zeros(C, np.float32)
    y = kernel(x, Wqkv, bqkv, Wout, bout)
    print("kernel ran:", y.shape, y.dtype, float(np.abs(y).max()))
